# revision 1
# baseline (speedup 1.0000x reference)
"""Transformer encoder layer (nn_Encoder) on 8 TRN2 NeuronCores.

Strategy: data-parallel over batch — B=8, one batch element per core, weights
replicated, no collectives. Per core a single Bass/Tile kernel computes the
whole layer; all large matmuls run in fp32r (full PE rate, ~1e-4 rel err).

Layout: attention runs in the "transposed domain" ([feature, tokens]) so every
weight matmul uses natural weight layouts; softmax over tokens-on-partitions is
handled by appending a ones-column to V (denominator lands in the ctx matmul's
extra output row, M=65). Per pair the 4 denominator rows are staged into one
[4, NS] tile, inverted with a single reciprocal, broadcast across partitions
with a tiny K=4 matmul against a selection matrix, and applied in one
full-width multiply per slice (deferred one pair to keep PE fed). Wo/FFN2
products land in the natural domain where both LayerNorms reduce along the
free dim (bn_stats). The only on-chip transposes are the 64 PE transposes of h
between LN1 and FFN1 (x arrives pre-transposed from the host).

Self-contained: hardcodes B=8, S=1024, D=1024, H=16, FF=2048, 8 cores.
"""
import math
import numpy as np
from contextlib import ExitStack

import concourse.bass as bass
import concourse.tile as tile
from concourse import bacc, mybir
from concourse import bass_utils
from concourse.masks import make_identity

B = 8
S = 1024
D = 1024
H = 16
FF = 2048
P = 128
HD = 64
EPS = 1e-5
f32 = mybir.dt.float32
f32r = mybir.dt.float32r
AF = mybir.ActivationFunctionType
ALU = mybir.AluOpType
AX = mybir.AxisListType

NP_ = H // 2          # head pairs
ST = S // P           # token tiles
DT = D // P
FT = FF // P
NS = 512              # token slice width (matmul free dim)
SL = S // NS
ND = 512              # feature slice width
DL = D // ND


def _layer_norm(nc, pool, v, out, g_b, be_b, si, pfx):
    """LayerNorm over the free dim of v [128, D] -> out = norm(v)*g + be.
    Sums computed on the Scalar engine (accum_out) which is idle in the
    Wo/FFN phases; DVE does only the small ops + 3 full-width passes."""
    scr = pool.tile([P, D], f32, name=f"{pfx}scr{si}", tag=f"{pfx}scr", bufs=3)
    st = pool.tile([P, 8], f32, name=f"{pfx}st{si}", tag=f"{pfx}st", bufs=4)
    s1 = st[:, 0:1]; s2 = st[:, 1:2]; mu = st[:, 2:3]; var = st[:, 3:4]
    sd = st[:, 4:5]; rstd = st[:, 5:6]
    nc.scalar.activation(scr[:], v[:], AF.Copy, accum_out=s1)
    nc.scalar.activation(scr[:], v[:], AF.Square, accum_out=s2)
    nc.vector.tensor_scalar_mul(mu, s1, 1.0 / D)
    nc.vector.tensor_scalar_mul(var, s2, 1.0 / D)
    nc.vector.tensor_mul(sd, mu, mu)
    nc.vector.tensor_sub(var, var, sd)
    nc.vector.tensor_scalar_add(var, var, EPS)
    nc.scalar.sqrt(sd, var)
    nc.vector.reciprocal(rstd, sd)
    nc.vector.tensor_scalar(out=v[:], in0=v[:], scalar1=mu, scalar2=rstd,
                            op0=ALU.subtract, op1=ALU.mult)
    nc.vector.tensor_mul(v[:], v[:], g_b[:])
    nc.vector.tensor_add(out[:], v[:], be_b[:])


def build_encoder(num_devices=8, exp_bufs=7):
    scale = 1.0 / math.sqrt(HD)
    nc = bacc.Bacc("TRN2", target_bir_lowering=False, debug=False,
                   enable_asserts=True, num_devices=num_devices)

    dram = lambda n, sh, dt: nc.dram_tensor(n, sh, dt, kind="ExternalInput").ap()
    xT_d = dram("xT", [D, S], f32r)
    vones_d = dram("vones", [P, H], f32r)
    sel_d = dram("sel", [SL, 4, P], f32r)
    x_d = dram("x", [S, D], f32)
    wq_d = dram("Wq", [NP_, DT, P, P], f32r)
    wk_d = dram("Wk", [NP_, DT, P, P], f32r)
    wv_d = dram("Wv", [D, D], f32r)
    wo_d = dram("Wo", [D, D], f32r)
    w1_d = dram("W1", [FT, DT, P, P], f32r)
    w2_d = dram("W2", [FF, D], f32r)
    bqc_d = dram("bqc", [P, NP_], f32)
    bkc_d = dram("bkc", [P, NP_], f32)
    b1c_d = dram("b1c", [P, FT], f32)
    bv_d = dram("bv", [D], f32)
    bo_d = dram("bo", [D], f32)
    b2_d = dram("b2", [D], f32)
    g1_d = dram("g1", [D], f32)
    be1_d = dram("be1", [D], f32)
    g2_d = dram("g2", [D], f32)
    be2_d = dram("be2", [D], f32)
    out_d = nc.dram_tensor("out", [S, D], f32, kind="ExternalOutput").ap()

    with tile.TileContext(nc) as tc, ExitStack() as octx:
        const = octx.enter_context(tc.tile_pool(name="const", bufs=1))
        identity = const.tile([P, P], f32, name="identity")
        make_identity(nc, identity)
        bqc = const.tile([P, NP_], f32, name="bqc")
        nc.sync.dma_start(bqc[:], bqc_d)
        bkc = const.tile([P, NP_], f32, name="bkc")
        nc.sync.dma_start(bkc[:], bkc_d)
        b1c = const.tile([P, FT], f32, name="b1c")
        nc.sync.dma_start(b1c[:], b1c_d)
        selt = const.tile([4, SL * P], f32r, name="selt")
        for sl in range(SL):
            nc.sync.dma_start(selt[:, sl * P:(sl + 1) * P], sel_d[sl])

        def bcast_row(pool, name, src_row, width):
            r = pool.tile([1, width], f32, name=f"{name}_r", tag="bcr", bufs=1)
            nc.sync.dma_start(r[:], src_row[None, :])
            b = pool.tile([P, width], f32, name=f"{name}_b", tag=f"{name}_b")
            nc.gpsimd.partition_broadcast(b[:], r[:])
            return b

        # ctxT pool (attention -> Wo; stays reserved to keep pool stack LIFO)
        pCtx = octx.enter_context(tc.tile_pool(name="pCtx", bufs=1))

        # ---------------- attention scope ----------------
        with tc.tile_pool(name="pA", bufs=1) as pA, \
             tc.tile_pool(name="psA", bufs=1, space="PSUM") as psA:

            # pair-0 Q/K weights first so QK(0) matmuls start ASAP
            wq0 = pA.tile([P, DT * P], f32r, name="wq0", tag="wq", bufs=2)
            nc.sync.dma_start(wq0[:].rearrange("p (dt q) -> p dt q", q=P),
                              wq_d[0].rearrange("dt dp q -> dp dt q"))
            wk0 = pA.tile([P, DT * P], f32r, name="wk0", tag="wk", bufs=2)
            nc.sync.dma_start(wk0[:].rearrange("p (dt q) -> p dt q", q=P),
                              wk_d[0].rearrange("dt dp q -> dp dt q"))

            xt = []
            for d in range(DT):
                t = pA.tile([P, S], f32r, name=f"xt{d}", tag="xt", bufs=DT)
                nc.sync.dma_start(t[:], xT_d[d * P:(d + 1) * P, :])
                xt.append(t)

            bv_b = bcast_row(pA, "bv", bv_d, D)

            # V65 tiles: [128 t, H*65] with ones columns at 65h+64
            v65 = []
            for t in range(ST):
                v = pA.tile([P, H * 65], f32r, name=f"v65_{t}", tag="v65",
                            bufs=ST)
                nc.sync.dma_start(
                    v.rearrange("p (h c) -> p h c", c=65)[:, :, 64:65],
                    vones_d[:, :, None])
                v65.append(v)

            pExp_cm = tc.tile_pool(name="pExp", bufs=1)
            pExp = pExp_cm.__enter__()

            # ---- V projection (wv pool; chunks emitted inside pair 0) ----
            pV_cm = tc.tile_pool(name="pV", bufs=1)
            pV = pV_cm.__enter__()
            wv = []
            for d in range(DT):
                t = pV.tile([P, D], f32r, name=f"wv{d}", tag="wv", bufs=DT)
                nc.sync.dma_start(t[:], wv_d[d * P:(d + 1) * P, :])
                wv.append(t)
            hpn = ND // HD
            v_state = {}

            def emit_v_chunk(hc):
                """Half-chunk hc of the V projection (chain = hc//2)."""
                chain = hc // 2
                part = hc % 2
                t, n = chain // DL, chain % DL
                if part == 0:
                    v_state[chain] = psA.tile(
                        [P, ND], f32, name=f"vps{t}_{n}", tag="vqk", bufs=2)
                ps = v_state[chain]
                for d in range(4 * part, 4 * part + 4):
                    nc.tensor.matmul(
                        ps[:], xt[d][:, t * P:(t + 1) * P],
                        wv[d][:, n * ND:(n + 1) * ND],
                        start=(d == 0), stop=(d == DT - 1))
                if part == 1:
                    dst = v65[t].rearrange("p (h c) -> p h c", c=65)[
                        :, n * hpn:(n + 1) * hpn, 0:64]
                    srcv = ps[:].rearrange("p (h k) -> p h k", k=HD)
                    bvs = bv_b[:, n * ND:(n + 1) * ND].rearrange(
                        "p (h k) -> p h k", k=HD)
                    nc.vector.tensor_add(dst, srcv, bvs)

            # ---- attention per head pair ----
            ctxT = [pCtx.tile([P, S], f32r, name=f"ctxT{p}", tag="ctxT",
                              bufs=NP_) for p in range(NP_)]

            def emit_normalize(p, ctxU, den4):
                """Deferred softmax-normalize of pair p's ctx."""
                den4r = pA.tile([4, NS], f32r, name=f"den4r_{p}", tag="den4r",
                                bufs=2)
                with nc.allow_low_precision("softmax denom recip in f32r"):
                    nc.vector.reciprocal(den4r[:], den4[:])
                for sl in range(SL):
                    rcb = psA.tile([P, NS], f32, name=f"rcb{p}_{sl}",
                                   tag="vqk", bufs=2)
                    nc.tensor.matmul(rcb[:], selt[:, sl * P:(sl + 1) * P],
                                     den4r[:], start=True, stop=True)
                    nc.vector.tensor_mul(
                        ctxT[p][:, sl * NS:(sl + 1) * NS],
                        ctxU[:, sl * NS:(sl + 1) * NS], rcb[:])

            def emit_qk_chain_part(p, chain, part, state):
                """Emit 4 of the 8 accumulation matmuls of QK chain
                (chain: 0..3 = Q-sl0, Q-sl1, K-sl0, K-sl1) for pair p."""
                wt, bc, dst = state["ops"][chain // 2]
                sl = chain % 2
                if part == 0:
                    state[chain] = psA.tile(
                        [P, NS], f32, name=f"qk{p}_{chain}", tag="vqk", bufs=2)
                ps = state[chain]
                for d in range(4 * part, 4 * part + 4):
                    nc.tensor.matmul(
                        ps[:], wt[:, d * P:(d + 1) * P],
                        xt[d][:, sl * NS:(sl + 1) * NS],
                        start=(d == 0), stop=(d == DT - 1))
                if part == 1:
                    nc.vector.tensor_scalar(
                        out=dst[:, sl * NS:(sl + 1) * NS], in0=ps[:],
                        scalar1=bc[:, p:p + 1], scalar2=None, op0=ALU.add)

            def make_qk_state(p):
                if p == 0:
                    wqt, wkt = wq0, wk0
                else:
                    wqt = pA.tile([P, DT * P], f32r, name=f"wq{p}", tag="wq",
                                  bufs=2)
                    nc.sync.dma_start(
                        wqt[:].rearrange("p (dt q) -> p dt q", q=P),
                        wq_d[p].rearrange("dt dp q -> dp dt q"))
                    wkt = pA.tile([P, DT * P], f32r, name=f"wk{p}", tag="wk",
                                  bufs=2)
                    nc.sync.dma_start(
                        wkt[:].rearrange("p (dt q) -> p dt q", q=P),
                        wk_d[p].rearrange("dt dp q -> dp dt q"))
                qt = pA.tile([P, S], f32r, name=f"qt{p}", tag="qt", bufs=2)
                kt = pA.tile([P, S], f32r, name=f"kt{p}", tag="kt", bufs=2)
                return {"ops": ((wqt, bqc, qt), (wkt, bkc, kt)),
                        "qt": qt, "kt": kt}

            LAG = 2
            qk_state = make_qk_state(0)
            for chain in range(4):
                for part in range(2):
                    emit_qk_chain_part(0, chain, part, qk_state)

            pending = None
            for p in range(NP_):
                qt, kt = qk_state["qt"], qk_state["kt"]
                next_state = make_qk_state(p + 1) if p + 1 < NP_ else None

                ctxU = pA.tile([P, S], f32, name=f"ctxU{p}", tag="ctxU",
                               bufs=2)
                den4 = pA.tile([4, NS], f32, name=f"den4_{p}", tag="den4",
                               bufs=2)

                def emit_scores(sl, t, expt):
                    ps = psA.tile([P, 2 * NS], f32, name=f"sc{t}_{sl}",
                                  tag="sc", bufs=2)
                    for h in range(2):
                        nc.tensor.matmul(
                            ps[:, h * NS:(h + 1) * NS],
                            kt[h * HD:(h + 1) * HD, t * P:(t + 1) * P],
                            qt[h * HD:(h + 1) * HD, sl * NS:(sl + 1) * NS],
                            start=True, stop=True,
                            tile_position=(h * HD, 0))
                    e = pExp.tile([P, 2 * NS], f32r, name=f"e{t}_{sl}",
                                  tag="exp", bufs=3)
                    nc.scalar.activation(e[:], ps[:], AF.Exp, scale=scale)
                    expt[t] = e

                def emit_ctx(sl, tt, cps, expt):
                    for h in range(2):
                        lhs = v65[tt][:, (2 * p + h) * 65:(2 * p + h) * 65 + 65]
                        nc.tensor.matmul(
                            cps[h][0:65, :], lhs,
                            expt[tt][:, h * NS:(h + 1) * NS],
                            start=(tt == 0), stop=(tt == ST - 1))

                def emit_evict(sl, cps):
                    for h in range(2):
                        ps = cps[h]
                        stage = pA.tile([65, NS], f32, name=f"stg{h}{sl}",
                                        tag="rc", bufs=2)
                        nc.vector.tensor_copy(stage[64:65, :], ps[64:65, :])
                        nc.sync.dma_start(
                            den4[h * SL + sl:h * SL + sl + 1, :],
                            stage[64:65, :])
                        if h == 0:
                            nc.vector.tensor_copy(
                                ctxU[0:HD, sl * NS:(sl + 1) * NS],
                                ps[0:HD, :])
                        else:
                            tmp = pA.tile([HD, NS], f32, name=f"ctmp{sl}",
                                          tag="ctmp", bufs=2)
                            nc.vector.tensor_copy(tmp[:], ps[0:HD, :])
                            nc.sync.dma_start(
                                ctxU[HD:P, sl * NS:(sl + 1) * NS], tmp[:])

                expt0 = {}
                cps0 = [psA.tile([P, NS], f32, name=f"cps{h}_0", tag="ctx",
                                 bufs=2) for h in range(2)]
                expt1 = {}
                cps1 = [psA.tile([P, NS], f32, name=f"cps{h}_1", tag="ctx",
                                 bufs=2) for h in range(2)]
                if p == 0:
                    # A: scores(sl0) + the whole V projection interleaved
                    for t in range(ST):
                        emit_scores(0, t, expt0)
                        for hc in range(4 * t, 4 * t + 4):
                            emit_v_chunk(hc)
                    # B: scores(sl1) + lagged ctx(sl0)
                    for t in range(ST + LAG):
                        if t < ST:
                            emit_scores(1, t, expt1)
                        if t >= LAG:
                            emit_ctx(0, t - LAG, cps0, expt0)
                    emit_evict(0, cps0)
                    # C: ctx(sl1) + QK(1) chunks
                    for t in range(ST):
                        emit_ctx(1, t, cps1, expt1)
                        if next_state is not None:
                            emit_qk_chain_part(p + 1, t // 2, t % 2,
                                               next_state)
                    emit_evict(1, cps1)
                    pV_cm.__exit__(None, None, None)
                else:
                    # A: scores(sl0) + QK(p+1) chunks 0-3 + lagged ctx(sl0)
                    for t in range(ST + LAG):
                        if t < ST:
                            emit_scores(0, t, expt0)
                            if next_state is not None and t < 4:
                                emit_qk_chain_part(p + 1, t // 2, t % 2,
                                                   next_state)
                        if t >= LAG:
                            emit_ctx(0, t - LAG, cps0, expt0)
                    emit_evict(0, cps0)
                    if pending is not None:
                        emit_normalize(*pending)
                    # B: scores(sl1) + QK(p+1) chunks 4-7 + lagged ctx(sl1)
                    for t in range(ST + LAG):
                        if t < ST:
                            emit_scores(1, t, expt1)
                            if next_state is not None and t < 4:
                                emit_qk_chain_part(p + 1, (t + 4) // 2,
                                                   t % 2, next_state)
                        if t >= LAG:
                            emit_ctx(1, t - LAG, cps1, expt1)
                    emit_evict(1, cps1)
                pending = (p, ctxU, den4)
                qk_state = next_state
            emit_normalize(*pending)
            pExp_cm.__exit__(None, None, None)

        # ---------------- Wo + LN1 scope ----------------
        pH = octx.enter_context(tc.tile_pool(name="pH", bufs=1))
        h_nat = []
        with tc.tile_pool(name="pWo", bufs=1) as pWo, \
             tc.tile_pool(name="psW", bufs=1, space="PSUM") as psW:
            bo_b = bcast_row(pWo, "bo", bo_d, D)
            g1_b = bcast_row(pWo, "g1", g1_d, D)
            be1_b = bcast_row(pWo, "be1", be1_d, D)

            wo = []
            for p in range(NP_):
                t = pWo.tile([P, D], f32r, name=f"wo{p}", tag="wo", bufs=NP_)
                nc.sync.dma_start(t[:], wo_d[p * P:(p + 1) * P, :])
                wo.append(t)

            ht = [pH.tile([P, S], f32r, name=f"ht{d}", tag="ht", bufs=DT)
                  for d in range(DT)]
            for si in range(ST):
                xn = pWo.tile([P, D], f32, name=f"xn{si}", tag="xn", bufs=4)
                nc.sync.dma_start(xn[:], x_d[si * P:(si + 1) * P, :])
                nc.vector.tensor_add(xn[:], xn[:], bo_b[:])   # fold bo into x
                pss = [psW.tile([P, ND], f32, name=f"c{si}_{n}", tag="c",
                                bufs=4) for n in range(DL)]
                for p in range(NP_):
                    for n in range(DL):
                        nc.tensor.matmul(
                            pss[n][:], ctxT[p][:, si * P:(si + 1) * P],
                            wo[p][:, n * ND:(n + 1) * ND],
                            start=(p == 0), stop=(p == NP_ - 1))
                v = pWo.tile([P, D], f32, name=f"v{si}", tag="v", bufs=4)
                for n in range(DL):
                    nc.vector.tensor_add(v[:, n * ND:(n + 1) * ND], pss[n][:],
                                         xn[:, n * ND:(n + 1) * ND])
                hn = pH.tile([P, D], f32, name=f"hn{si}", tag="hn", bufs=ST)
                _layer_norm(nc, pWo, v, hn, g1_b, be1_b, si, "ln1")
                h_nat.append(hn)
                # h^T transposes for this si, interleaved with the next si's
                # Wo matmuls; copybacks alternate DVE/ACT
                for dd in range(DT):
                    ps = psW.tile([P, P], f32, name=f"tp{si}_{dd}", tag="tp",
                                  bufs=4)
                    nc.tensor.transpose(
                        ps[:], hn[:, dd * P:(dd + 1) * P], identity[:])
                    dst = ht[dd][:, si * P:(si + 1) * P]
                    if dd % 2 == 0:
                        nc.vector.tensor_copy(dst, ps[:])
                    else:
                        nc.scalar.copy(dst, ps[:])

        # ---------------- FFN + LN2 scope ----------------
        # Split over s-halves so uT is half-resident; W1/W2 streamed per half.
        with tc.tile_pool(name="pF", bufs=1) as pF:
            b2_b = bcast_row(pF, "b2", b2_d, D)
            g2_b = bcast_row(pF, "g2", g2_d, D)
            be2_b = bcast_row(pF, "be2", be2_d, D)

            for half in range(S // NS):
                s0 = half * NS
                # FFN1 + relu for this s-half
                ut = []
                with tc.tile_pool(name=f"psU{half}", bufs=1,
                                  space="PSUM") as psU:
                    for f in range(FT):
                        w1t = pF.tile([P, DT * P], f32r, name=f"w1_{half}_{f}",
                                      tag="w1", bufs=3)
                        nc.sync.dma_start(
                            w1t[:].rearrange("p (dt q) -> p dt q", q=P),
                            w1_d[f].rearrange("dt dp q -> dp dt q"))
                        u = pF.tile([P, NS], f32r, name=f"ut{half}_{f}",
                                    tag="ut", bufs=FT)
                        ps = psU.tile([P, NS], f32, name=f"u{half}_{f}",
                                      tag="u", bufs=3)
                        for d in range(DT):
                            nc.tensor.matmul(
                                ps[:], w1t[:, d * P:(d + 1) * P],
                                ht[d][:, s0:s0 + NS],
                                start=(d == 0), stop=(d == DT - 1))
                        nc.scalar.activation(u[:], ps[:], AF.Relu,
                                             bias=b1c[:, f:f + 1])
                        ut.append(u)

                # FFN2 + LN2 + out: one 4-si group (8 y-psum banks), W2
                # streamed once per half
                si0 = half * (ST // 2)
                with tc.tile_pool(name=f"psY{half}", bufs=1,
                                  space="PSUM") as psY:
                    sis = range(si0, si0 + ST // 2)
                    pss = {(si, n): psY.tile([P, ND], f32,
                                             name=f"y{si}_{n}", tag="y",
                                             bufs=8)
                           for si in sis for n in range(DL)}
                    for f in range(FT):
                        w2t = pF.tile([P, D], f32r, name=f"w2_{half}_{f}",
                                      tag="w2", bufs=3)
                        nc.sync.dma_start(w2t[:], w2_d[f * P:(f + 1) * P, :])
                        for si in sis:
                            for n in range(DL):
                                nc.tensor.matmul(
                                    pss[(si, n)][:],
                                    ut[f][:, (si % (ST // 2)) * P:
                                          (si % (ST // 2) + 1) * P],
                                    w2t[:, n * ND:(n + 1) * ND],
                                    start=(f == 0), stop=(f == FT - 1))
                    for si in sis:
                        v = pF.tile([P, D], f32, name=f"v2_{si}", tag="v2",
                                    bufs=3)
                        for n in range(DL):
                            nc.vector.tensor_add(
                                v[:, n * ND:(n + 1) * ND], pss[(si, n)][:],
                                h_nat[si][:, n * ND:(n + 1) * ND])
                        nc.vector.tensor_add(v[:], v[:], b2_b[:])
                        o = pF.tile([P, D], f32, name=f"o{si}", tag="o",
                                    bufs=3)
                        _layer_norm(nc, pF, v, o, g2_b, be2_b, si, "ln2")
                        nc.sync.dma_start(out_d[si * P:(si + 1) * P, :], o[:])

    nc.compile()
    return nc


def pack_core_inputs(x_b, shared):
    """Per-core input map: batch element x_b + shared (prepacked) weights."""
    m = dict(shared)
    m["x"] = np.ascontiguousarray(x_b, dtype=np.float32)
    m["xT"] = np.ascontiguousarray(x_b.T, dtype=np.float32)
    return m


def pack_shared(Wq, bq, Wk, bk, Wv, bv, Wo, bo, ln1_g, ln1_b, W1, b1, W2, b2,
                ln2_g, ln2_b):
    """Host-side layout packing of the replicated weights (pure layout)."""
    f = np.float32
    Wq = np.asarray(Wq, dtype=f); Wk = np.asarray(Wk, dtype=f)
    Wv = np.asarray(Wv, dtype=f)
    pack_qk = lambda W: np.ascontiguousarray(
        W.reshape(D, H * HD).reshape(DT, P, NP_, P).transpose(2, 0, 1, 3))
    sel = np.zeros((SL, 4, P), dtype=f)
    for sl in range(SL):
        for m in range(P):
            sel[sl, (m // HD) * SL + sl, m] = 1.0
    return {
        "vones": np.ones((P, H), dtype=f),
        "sel": sel,
        "Wq": pack_qk(Wq), "Wk": pack_qk(Wk),
        "Wv": np.ascontiguousarray(Wv.reshape(D, D)),
        "Wo": np.ascontiguousarray(Wo, dtype=f),
        "W1": np.ascontiguousarray(
            np.asarray(W1, dtype=f).reshape(DT, P, FT, P).transpose(
                2, 0, 1, 3)),
        "W2": np.ascontiguousarray(W2, dtype=f),
        "bqc": np.ascontiguousarray(np.asarray(bq, f).reshape(NP_, P).T),
        "bkc": np.ascontiguousarray(np.asarray(bk, f).reshape(NP_, P).T),
        "b1c": np.ascontiguousarray(np.asarray(b1, f).reshape(FT, P).T),
        "bv": np.ascontiguousarray(np.asarray(bv, f).reshape(D)),
        "bo": np.ascontiguousarray(bo, dtype=f),
        "b2": np.ascontiguousarray(b2, dtype=f),
        "g1": np.ascontiguousarray(ln1_g, dtype=f),
        "be1": np.ascontiguousarray(ln1_b, dtype=f),
        "g2": np.ascontiguousarray(ln2_g, dtype=f),
        "be2": np.ascontiguousarray(ln2_b, dtype=f),
    }


_NC_CACHE = {}


def get_nc():
    if "nc" not in _NC_CACHE:
        _NC_CACHE["nc"] = build_encoder(num_devices=8)
    return _NC_CACHE["nc"]


def kernel(x, Wq, bq, Wk, bk, Wv, bv, Wo, bo, ln1_g, ln1_b, W1, b1, W2, b2,
           ln2_g, ln2_b):
    x = np.asarray(x)
    assert x.shape == (B, S, D)
    shared = pack_shared(Wq, bq, Wk, bk, Wv, bv, Wo, bo, ln1_g, ln1_b,
                         W1, b1, W2, b2, ln2_g, ln2_b)
    in_maps = [pack_core_inputs(x[b], shared) for b in range(B)]
    nc = get_nc()
    res = bass_utils.run_bass_kernel_spmd(
        nc, in_maps, core_ids=list(range(B)), trace=False)
    return np.stack([res.results[b]["out"] for b in range(B)], axis=0)



# revision 8
# speedup vs baseline: 1.2029x; 1.2029x over previous
"""Transformer encoder layer (nn_Encoder) on 8 TRN2 NeuronCores.

Strategy: data-parallel over batch — B=8, one batch element per core, weights
replicated, no collectives. Per core a single Bass/Tile kernel computes the
whole layer; all large matmuls run in fp32r (full PE rate, ~1e-4 rel err).

Layout: attention runs in the "transposed domain" ([feature, tokens]) so every
weight matmul uses natural weight layouts; softmax over tokens-on-partitions is
handled by appending a ones-column to V (denominator lands in the ctx matmul's
extra output row, M=65). Per pair the 4 denominator rows are staged into one
[4, NS] tile, inverted with a single reciprocal, broadcast across partitions
with a tiny K=4 matmul against a selection matrix, and applied in one
full-width multiply per slice (deferred one pair to keep PE fed). Wo/FFN2
products land in the natural domain where both LayerNorms reduce along the
free dim (bn_stats). The only on-chip transposes are the 64 PE transposes of h
between LN1 and FFN1 (x arrives pre-transposed from the host).

Self-contained: hardcodes B=8, S=1024, D=1024, H=16, FF=2048, 8 cores.
"""
import math
import numpy as np
import ml_dtypes
from contextlib import ExitStack

import concourse.bass as bass
import concourse.tile as tile
from concourse import bacc, mybir
from concourse import bass_utils
from concourse.masks import make_identity

B = 8
S = 1024
D = 1024
H = 16
FF = 2048
P = 128
HD = 64
EPS = 1e-5
f32 = mybir.dt.float32
f32r = mybir.dt.float32r
bf16 = mybir.dt.bfloat16
np_bf16 = ml_dtypes.bfloat16
AF = mybir.ActivationFunctionType
ALU = mybir.AluOpType
AX = mybir.AxisListType

NP_ = H // 2          # head pairs
ST = S // P           # token tiles
DT = D // P
FT = FF // P
NS = 512              # token slice width (matmul free dim)
SL = S // NS
ND = 512              # feature slice width
DL = D // ND


def _layer_norm(nc, pool, v, out, g_b, be_b, si, pfx):
    """LayerNorm over the free dim of v [128, D] -> out = norm(v)*g + be.
    Sums computed on the Scalar engine (accum_out) which is idle in the
    Wo/FFN phases; DVE does only the small ops + 3 full-width passes."""
    scr = pool.tile([P, D], f32, name=f"{pfx}scr{si}", tag=f"{pfx}scr", bufs=3)
    st = pool.tile([P, 8], f32, name=f"{pfx}st{si}", tag=f"{pfx}st", bufs=4)
    s1 = st[:, 0:1]; s2 = st[:, 1:2]; mu = st[:, 2:3]; var = st[:, 3:4]
    sd = st[:, 4:5]; rstd = st[:, 5:6]
    nc.scalar.activation(scr[:], v[:], AF.Copy, accum_out=s1)
    nc.scalar.activation(scr[:], v[:], AF.Square, accum_out=s2)
    nc.vector.tensor_scalar_mul(mu, s1, 1.0 / D)
    nc.vector.tensor_scalar_mul(var, s2, 1.0 / D)
    nc.vector.tensor_mul(sd, mu, mu)
    nc.vector.tensor_sub(var, var, sd)
    nc.vector.tensor_scalar_add(var, var, EPS)
    nc.scalar.sqrt(sd, var)
    nc.vector.reciprocal(rstd, sd)
    nc.vector.tensor_scalar(out=v[:], in0=v[:], scalar1=mu, scalar2=rstd,
                            op0=ALU.subtract, op1=ALU.mult)
    nc.vector.tensor_mul(v[:], v[:], g_b[:])
    nc.vector.tensor_add(out[:], v[:], be_b[:])


def build_encoder(num_devices=8, exp_bufs=7):
    scale = 1.0 / math.sqrt(HD)
    nc = bacc.Bacc("TRN2", target_bir_lowering=False, debug=False,
                   enable_asserts=True, num_devices=num_devices)

    dram = lambda n, sh, dt: nc.dram_tensor(n, sh, dt, kind="ExternalInput").ap()
    xT_d = dram("xT", [D, S], bf16)
    vones_d = dram("vones", [P, H], bf16)
    sel_d = dram("sel", [SL, 4, P], f32r)
    x_d = dram("x", [S, D], f32)
    wq_d = dram("Wq", [NP_, DT, P, P], bf16)
    wk_d = dram("Wk", [NP_, DT, P, P], bf16)
    wv_d = dram("Wv", [D, D], bf16)
    wo_d = dram("Wo", [D, D], bf16)
    w1_d = dram("W1", [FT, DT, P, P], bf16)
    w2_d = dram("W2", [FF, D], bf16)
    bqc_d = dram("bqc", [P, NP_], f32)
    bkc_d = dram("bkc", [P, NP_], f32)
    b1c_d = dram("b1c", [P, FT], f32)
    bv_d = dram("bv", [D], f32)
    bo_d = dram("bo", [D], f32)
    b2_d = dram("b2", [D], f32)
    g1_d = dram("g1", [D], f32)
    be1_d = dram("be1", [D], f32)
    g2_d = dram("g2", [D], f32)
    be2_d = dram("be2", [D], f32)
    out_d = nc.dram_tensor("out", [S, D], f32, kind="ExternalOutput").ap()

    with tile.TileContext(nc) as tc, ExitStack() as octx:
        const = octx.enter_context(tc.tile_pool(name="const", bufs=1))
        identity = const.tile([P, P], bf16, name="identity")
        make_identity(nc, identity)
        bqc = const.tile([P, NP_], f32, name="bqc")
        nc.sync.dma_start(bqc[:], bqc_d)
        bkc = const.tile([P, NP_], f32, name="bkc")
        nc.sync.dma_start(bkc[:], bkc_d)
        b1c = const.tile([P, FT], f32, name="b1c")
        nc.sync.dma_start(b1c[:], b1c_d)
        selt = const.tile([4, SL * P], f32r, name="selt")
        for sl in range(SL):
            nc.sync.dma_start(selt[:, sl * P:(sl + 1) * P], sel_d[sl])

        def bcast_row(pool, name, src_row, width):
            r = pool.tile([1, width], f32, name=f"{name}_r", tag="bcr", bufs=1)
            nc.sync.dma_start(r[:], src_row[None, :])
            b = pool.tile([P, width], f32, name=f"{name}_b", tag=f"{name}_b")
            nc.gpsimd.partition_broadcast(b[:], r[:])
            return b

        # ctxT pool (attention -> Wo; stays reserved to keep pool stack LIFO)
        pCtx = octx.enter_context(tc.tile_pool(name="pCtx", bufs=1))

        # ---------------- attention scope ----------------
        with tc.tile_pool(name="pA", bufs=1) as pA, \
             tc.tile_pool(name="psA", bufs=1, space="PSUM") as psA:

            # pair-0 Q/K weights first so QK(0) matmuls start ASAP
            wq0 = pA.tile([P, DT * P], bf16, name="wq0", tag="wq", bufs=2)
            nc.sync.dma_start(wq0[:].rearrange("p (dt q) -> p dt q", q=P),
                              wq_d[0].rearrange("dt dp q -> dp dt q"))
            wk0 = pA.tile([P, DT * P], bf16, name="wk0", tag="wk", bufs=2)
            nc.sync.dma_start(wk0[:].rearrange("p (dt q) -> p dt q", q=P),
                              wk_d[0].rearrange("dt dp q -> dp dt q"))

            xt = []
            for d in range(DT):
                t = pA.tile([P, S], bf16, name=f"xt{d}", tag="xt", bufs=DT)
                nc.sync.dma_start(t[:], xT_d[d * P:(d + 1) * P, :])
                xt.append(t)

            bv_b = bcast_row(pA, "bv", bv_d, D)

            # V65 tiles: [128 t, H*65] with ones columns at 65h+64
            v65 = []
            for t in range(ST):
                v = pA.tile([P, H * 65], bf16, name=f"v65_{t}", tag="v65",
                            bufs=ST)
                nc.sync.dma_start(
                    v.rearrange("p (h c) -> p h c", c=65)[:, :, 64:65],
                    vones_d[:, :, None])
                v65.append(v)

            pExp_cm = tc.tile_pool(name="pExp", bufs=1)
            pExp = pExp_cm.__enter__()

            # ---- V projection (wv pool; chunks emitted inside pair 0) ----
            pV_cm = tc.tile_pool(name="pV", bufs=1)
            pV = pV_cm.__enter__()
            wv = []
            for d in range(DT):
                t = pV.tile([P, D], bf16, name=f"wv{d}", tag="wv", bufs=DT)
                nc.sync.dma_start(t[:], wv_d[d * P:(d + 1) * P, :])
                wv.append(t)
            hpn = ND // HD
            v_state = {}

            def emit_v_chunk(hc):
                """Half-chunk hc of the V projection (chain = hc//2)."""
                chain = hc // 2
                part = hc % 2
                t, n = chain // DL, chain % DL
                if part == 0:
                    v_state[chain] = psA.tile(
                        [P, ND], f32, name=f"vps{t}_{n}", tag="vqk", bufs=2)
                ps = v_state[chain]
                for d in range(4 * part, 4 * part + 4):
                    nc.tensor.matmul(
                        ps[:], xt[d][:, t * P:(t + 1) * P],
                        wv[d][:, n * ND:(n + 1) * ND],
                        start=(d == 0), stop=(d == DT - 1))
                if part == 1:
                    dst = v65[t].rearrange("p (h c) -> p h c", c=65)[
                        :, n * hpn:(n + 1) * hpn, 0:64]
                    srcv = ps[:].rearrange("p (h k) -> p h k", k=HD)
                    bvs = bv_b[:, n * ND:(n + 1) * ND].rearrange(
                        "p (h k) -> p h k", k=HD)
                    nc.vector.tensor_add(dst, srcv, bvs)

            # ---- attention per head pair ----
            ctxT = [pCtx.tile([P, S], bf16, name=f"ctxT{p}", tag="ctxT",
                              bufs=NP_) for p in range(NP_)]

            def emit_normalize(p, ctxU, den4):
                """Deferred softmax-normalize of pair p's ctx."""
                den4r = pA.tile([4, NS], f32r, name=f"den4r_{p}", tag="den4r",
                                bufs=2)
                with nc.allow_low_precision("softmax denom recip in f32r"):
                    nc.vector.reciprocal(den4r[:], den4[:])
                for sl in range(SL):
                    rcb = psA.tile([P, NS], f32, name=f"rcb{p}_{sl}",
                                   tag="vqk", bufs=2)
                    nc.tensor.matmul(rcb[:], selt[:, sl * P:(sl + 1) * P],
                                     den4r[:], start=True, stop=True)
                    nc.vector.tensor_mul(
                        ctxT[p][:, sl * NS:(sl + 1) * NS],
                        ctxU[:, sl * NS:(sl + 1) * NS], rcb[:])

            def emit_qk_chain_part(p, chain, part, state):
                """Emit 4 of the 8 accumulation matmuls of QK chain
                (chain: 0..3 = Q-sl0, Q-sl1, K-sl0, K-sl1) for pair p."""
                wt, bc, dst = state["ops"][chain // 2]
                sl = chain % 2
                if part == 0:
                    state[chain] = psA.tile(
                        [P, NS], f32, name=f"qk{p}_{chain}", tag="vqk", bufs=2)
                ps = state[chain]
                for d in range(4 * part, 4 * part + 4):
                    nc.tensor.matmul(
                        ps[:], wt[:, d * P:(d + 1) * P],
                        xt[d][:, sl * NS:(sl + 1) * NS],
                        start=(d == 0), stop=(d == DT - 1))
                if part == 1:
                    nc.vector.tensor_scalar(
                        out=dst[:, sl * NS:(sl + 1) * NS], in0=ps[:],
                        scalar1=bc[:, p:p + 1], scalar2=None, op0=ALU.add)

            def make_qk_state(p):
                if p == 0:
                    wqt, wkt = wq0, wk0
                else:
                    wqt = pA.tile([P, DT * P], bf16, name=f"wq{p}", tag="wq",
                                  bufs=2)
                    nc.sync.dma_start(
                        wqt[:].rearrange("p (dt q) -> p dt q", q=P),
                        wq_d[p].rearrange("dt dp q -> dp dt q"))
                    wkt = pA.tile([P, DT * P], bf16, name=f"wk{p}", tag="wk",
                                  bufs=2)
                    nc.sync.dma_start(
                        wkt[:].rearrange("p (dt q) -> p dt q", q=P),
                        wk_d[p].rearrange("dt dp q -> dp dt q"))
                qt = pA.tile([P, S], bf16, name=f"qt{p}", tag="qt", bufs=2)
                kt = pA.tile([P, S], bf16, name=f"kt{p}", tag="kt", bufs=2)
                return {"ops": ((wqt, bqc, qt), (wkt, bkc, kt)),
                        "qt": qt, "kt": kt}

            LAG = 2
            qk_state = make_qk_state(0)
            for chain in range(4):
                for part in range(2):
                    emit_qk_chain_part(0, chain, part, qk_state)

            pending = None
            for p in range(NP_):
                qt, kt = qk_state["qt"], qk_state["kt"]
                next_state = make_qk_state(p + 1) if p + 1 < NP_ else None

                ctxU = pA.tile([P, S], f32, name=f"ctxU{p}", tag="ctxU",
                               bufs=2)
                den4 = pA.tile([4, NS], f32, name=f"den4_{p}", tag="den4",
                               bufs=2)

                def emit_scores(sl, t, expt):
                    ps = psA.tile([P, 2 * NS], f32, name=f"sc{t}_{sl}",
                                  tag="sc", bufs=2)
                    for h in range(2):
                        nc.tensor.matmul(
                            ps[:, h * NS:(h + 1) * NS],
                            kt[h * HD:(h + 1) * HD, t * P:(t + 1) * P],
                            qt[h * HD:(h + 1) * HD, sl * NS:(sl + 1) * NS],
                            start=True, stop=True,
                            tile_position=(h * HD, 0))
                    e = pExp.tile([P, 2 * NS], bf16, name=f"e{t}_{sl}",
                                  tag="exp", bufs=3)
                    nc.scalar.activation(e[:], ps[:], AF.Exp, scale=scale)
                    expt[t] = e

                def emit_ctx(sl, tt, cps, expt):
                    for h in range(2):
                        lhs = v65[tt][:, (2 * p + h) * 65:(2 * p + h) * 65 + 65]
                        nc.tensor.matmul(
                            cps[h][0:65, :], lhs,
                            expt[tt][:, h * NS:(h + 1) * NS],
                            start=(tt == 0), stop=(tt == ST - 1))

                def emit_evict(sl, cps):
                    for h in range(2):
                        ps = cps[h]
                        stage = pA.tile([65, NS], f32, name=f"stg{h}{sl}",
                                        tag="rc", bufs=2)
                        nc.vector.tensor_copy(stage[64:65, :], ps[64:65, :])
                        nc.sync.dma_start(
                            den4[h * SL + sl:h * SL + sl + 1, :],
                            stage[64:65, :])
                        if h == 0:
                            nc.vector.tensor_copy(
                                ctxU[0:HD, sl * NS:(sl + 1) * NS],
                                ps[0:HD, :])
                        else:
                            tmp = pA.tile([HD, NS], f32, name=f"ctmp{sl}",
                                          tag="ctmp", bufs=2)
                            nc.vector.tensor_copy(tmp[:], ps[0:HD, :])
                            nc.sync.dma_start(
                                ctxU[HD:P, sl * NS:(sl + 1) * NS], tmp[:])

                expt0 = {}
                cps0 = [psA.tile([P, NS], f32, name=f"cps{h}_0", tag="ctx",
                                 bufs=2) for h in range(2)]
                expt1 = {}
                cps1 = [psA.tile([P, NS], f32, name=f"cps{h}_1", tag="ctx",
                                 bufs=2) for h in range(2)]
                if p == 0:
                    # A: scores(sl0) + the whole V projection interleaved
                    for t in range(ST):
                        emit_scores(0, t, expt0)
                        for hc in range(4 * t, 4 * t + 4):
                            emit_v_chunk(hc)
                    # B: scores(sl1) + lagged ctx(sl0)
                    for t in range(ST + LAG):
                        if t < ST:
                            emit_scores(1, t, expt1)
                        if t >= LAG:
                            emit_ctx(0, t - LAG, cps0, expt0)
                    emit_evict(0, cps0)
                    # C: ctx(sl1) + QK(1) chunks
                    for t in range(ST):
                        emit_ctx(1, t, cps1, expt1)
                        if next_state is not None:
                            emit_qk_chain_part(p + 1, t // 2, t % 2,
                                               next_state)
                    emit_evict(1, cps1)
                    pV_cm.__exit__(None, None, None)
                else:
                    # A: scores(sl0) + QK(p+1) chunks 0-3 + lagged ctx(sl0)
                    for t in range(ST + LAG):
                        if t < ST:
                            emit_scores(0, t, expt0)
                            if next_state is not None and t < 4:
                                emit_qk_chain_part(p + 1, t // 2, t % 2,
                                                   next_state)
                        if t >= LAG:
                            emit_ctx(0, t - LAG, cps0, expt0)
                    emit_evict(0, cps0)
                    if pending is not None:
                        emit_normalize(*pending)
                    # B: scores(sl1) + QK(p+1) chunks 4-7 + lagged ctx(sl1)
                    for t in range(ST + LAG):
                        if t < ST:
                            emit_scores(1, t, expt1)
                            if next_state is not None and t < 4:
                                emit_qk_chain_part(p + 1, (t + 4) // 2,
                                                   t % 2, next_state)
                        if t >= LAG:
                            emit_ctx(1, t - LAG, cps1, expt1)
                    emit_evict(1, cps1)
                pending = (p, ctxU, den4)
                qk_state = next_state
            emit_normalize(*pending)
            pExp_cm.__exit__(None, None, None)

        # ---------------- Wo + LN1 scope ----------------
        pH = octx.enter_context(tc.tile_pool(name="pH", bufs=1))
        h_nat = []
        with tc.tile_pool(name="pWo", bufs=1) as pWo, \
             tc.tile_pool(name="psW", bufs=1, space="PSUM") as psW:
            bo_b = bcast_row(pWo, "bo", bo_d, D)
            g1_b = bcast_row(pWo, "g1", g1_d, D)
            be1_b = bcast_row(pWo, "be1", be1_d, D)

            wo = []
            for p in range(NP_):
                t = pWo.tile([P, D], bf16, name=f"wo{p}", tag="wo", bufs=NP_)
                nc.sync.dma_start(t[:], wo_d[p * P:(p + 1) * P, :])
                wo.append(t)

            ht = [pH.tile([P, S], bf16, name=f"ht{d}", tag="ht", bufs=DT)
                  for d in range(DT)]
            for si in range(ST):
                xn = pWo.tile([P, D], f32, name=f"xn{si}", tag="xn", bufs=4)
                nc.sync.dma_start(xn[:], x_d[si * P:(si + 1) * P, :])
                nc.vector.tensor_add(xn[:], xn[:], bo_b[:])   # fold bo into x
                pss = [psW.tile([P, ND], f32, name=f"c{si}_{n}", tag="c",
                                bufs=4) for n in range(DL)]
                for p in range(NP_):
                    for n in range(DL):
                        nc.tensor.matmul(
                            pss[n][:], ctxT[p][:, si * P:(si + 1) * P],
                            wo[p][:, n * ND:(n + 1) * ND],
                            start=(p == 0), stop=(p == NP_ - 1))
                v = pWo.tile([P, D], f32, name=f"v{si}", tag="v", bufs=4)
                for n in range(DL):
                    nc.vector.tensor_add(v[:, n * ND:(n + 1) * ND], pss[n][:],
                                         xn[:, n * ND:(n + 1) * ND])
                hn = pH.tile([P, D], bf16, name=f"hn{si}", tag="hn", bufs=ST)
                _layer_norm(nc, pWo, v, hn, g1_b, be1_b, si, "ln1")
                h_nat.append(hn)
                # h^T transposes for this si, interleaved with the next si's
                # Wo matmuls; copybacks alternate DVE/ACT
                for dd in range(DT):
                    ps = psW.tile([P, P], bf16, name=f"tp{si}_{dd}", tag="tp",
                                  bufs=4)
                    nc.tensor.transpose(
                        ps[:], hn[:, dd * P:(dd + 1) * P], identity[:])
                    dst = ht[dd][:, si * P:(si + 1) * P]
                    if dd % 2 == 0:
                        nc.vector.tensor_copy(dst, ps[:])
                    else:
                        nc.scalar.copy(dst, ps[:])

        # ---------------- FFN + LN2 scope ----------------
        # Split over s-halves so uT is half-resident; W1/W2 streamed per half.
        with tc.tile_pool(name="pF", bufs=1) as pF:
            b2_b = bcast_row(pF, "b2", b2_d, D)
            g2_b = bcast_row(pF, "g2", g2_d, D)
            be2_b = bcast_row(pF, "be2", be2_d, D)

            for half in range(S // NS):
                s0 = half * NS
                # FFN1 + relu for this s-half
                ut = []
                with tc.tile_pool(name=f"psU{half}", bufs=1,
                                  space="PSUM") as psU:
                    for f in range(FT):
                        w1t = pF.tile([P, DT * P], bf16, name=f"w1_{half}_{f}",
                                      tag="w1", bufs=3)
                        nc.sync.dma_start(
                            w1t[:].rearrange("p (dt q) -> p dt q", q=P),
                            w1_d[f].rearrange("dt dp q -> dp dt q"))
                        u = pF.tile([P, NS], bf16, name=f"ut{half}_{f}",
                                    tag="ut", bufs=FT)
                        ps = psU.tile([P, NS], f32, name=f"u{half}_{f}",
                                      tag="u", bufs=3)
                        for d in range(DT):
                            nc.tensor.matmul(
                                ps[:], w1t[:, d * P:(d + 1) * P],
                                ht[d][:, s0:s0 + NS],
                                start=(d == 0), stop=(d == DT - 1))
                        nc.scalar.activation(u[:], ps[:], AF.Relu,
                                             bias=b1c[:, f:f + 1])
                        ut.append(u)

                # FFN2 + LN2 + out: one 4-si group (8 y-psum banks), W2
                # streamed once per half
                si0 = half * (ST // 2)
                with tc.tile_pool(name=f"psY{half}", bufs=1,
                                  space="PSUM") as psY:
                    sis = range(si0, si0 + ST // 2)
                    pss = {(si, n): psY.tile([P, ND], f32,
                                             name=f"y{si}_{n}", tag="y",
                                             bufs=8)
                           for si in sis for n in range(DL)}
                    for f in range(FT):
                        w2t = pF.tile([P, D], bf16, name=f"w2_{half}_{f}",
                                      tag="w2", bufs=3)
                        nc.sync.dma_start(w2t[:], w2_d[f * P:(f + 1) * P, :])
                        for si in sis:
                            for n in range(DL):
                                nc.tensor.matmul(
                                    pss[(si, n)][:],
                                    ut[f][:, (si % (ST // 2)) * P:
                                          (si % (ST // 2) + 1) * P],
                                    w2t[:, n * ND:(n + 1) * ND],
                                    start=(f == 0), stop=(f == FT - 1))
                    for si in sis:
                        v = pF.tile([P, D], f32, name=f"v2_{si}", tag="v2",
                                    bufs=3)
                        for n in range(DL):
                            nc.vector.tensor_add(
                                v[:, n * ND:(n + 1) * ND], pss[(si, n)][:],
                                h_nat[si][:, n * ND:(n + 1) * ND])
                        nc.vector.tensor_add(v[:], v[:], b2_b[:])
                        o = pF.tile([P, D], f32, name=f"o{si}", tag="o",
                                    bufs=3)
                        _layer_norm(nc, pF, v, o, g2_b, be2_b, si, "ln2")
                        nc.sync.dma_start(out_d[si * P:(si + 1) * P, :], o[:])

    nc.compile()
    return nc


def pack_core_inputs(x_b, shared):
    """Per-core input map: batch element x_b + shared (prepacked) weights."""
    m = dict(shared)
    m["x"] = np.ascontiguousarray(x_b, dtype=np.float32)
    m["xT"] = np.ascontiguousarray(x_b.T.astype(np_bf16))
    return m


def pack_shared(Wq, bq, Wk, bk, Wv, bv, Wo, bo, ln1_g, ln1_b, W1, b1, W2, b2,
                ln2_g, ln2_b):
    """Host-side layout packing of the replicated weights (pure layout)."""
    f = np.float32
    Wq = np.asarray(Wq, dtype=f); Wk = np.asarray(Wk, dtype=f)
    Wv = np.asarray(Wv, dtype=f)
    pack_qk = lambda W: np.ascontiguousarray(
        W.reshape(D, H * HD).reshape(DT, P, NP_, P).transpose(2, 0, 1, 3))
    sel = np.zeros((SL, 4, P), dtype=f)
    for sl in range(SL):
        for m in range(P):
            sel[sl, (m // HD) * SL + sl, m] = 1.0
    return {
        "vones": np.ones((P, H), dtype=np_bf16),
        "sel": sel,
        "Wq": pack_qk(Wq).astype(np_bf16), "Wk": pack_qk(Wk).astype(np_bf16),
        "Wv": np.ascontiguousarray(Wv.reshape(D, D)).astype(np_bf16),
        "Wo": np.ascontiguousarray(Wo, dtype=f).astype(np_bf16),
        "W1": np.ascontiguousarray(
            np.asarray(W1, dtype=f).reshape(DT, P, FT, P).transpose(
                2, 0, 1, 3)).astype(np_bf16),
        "W2": np.ascontiguousarray(W2, dtype=f).astype(np_bf16),
        "bqc": np.ascontiguousarray(np.asarray(bq, f).reshape(NP_, P).T),
        "bkc": np.ascontiguousarray(np.asarray(bk, f).reshape(NP_, P).T),
        "b1c": np.ascontiguousarray(np.asarray(b1, f).reshape(FT, P).T),
        "bv": np.ascontiguousarray(np.asarray(bv, f).reshape(D)),
        "bo": np.ascontiguousarray(bo, dtype=f),
        "b2": np.ascontiguousarray(b2, dtype=f),
        "g1": np.ascontiguousarray(ln1_g, dtype=f),
        "be1": np.ascontiguousarray(ln1_b, dtype=f),
        "g2": np.ascontiguousarray(ln2_g, dtype=f),
        "be2": np.ascontiguousarray(ln2_b, dtype=f),
    }


_NC_CACHE = {}


def get_nc():
    if "nc" not in _NC_CACHE:
        _NC_CACHE["nc"] = build_encoder(num_devices=8)
    return _NC_CACHE["nc"]


def kernel(x, Wq, bq, Wk, bk, Wv, bv, Wo, bo, ln1_g, ln1_b, W1, b1, W2, b2,
           ln2_g, ln2_b):
    x = np.asarray(x)
    assert x.shape == (B, S, D)
    shared = pack_shared(Wq, bq, Wk, bk, Wv, bv, Wo, bo, ln1_g, ln1_b,
                         W1, b1, W2, b2, ln2_g, ln2_b)
    in_maps = [pack_core_inputs(x[b], shared) for b in range(B)]
    nc = get_nc()
    res = bass_utils.run_bass_kernel_spmd(
        nc, in_maps, core_ids=list(range(B)), trace=False)
    return np.stack([res.results[b]["out"] for b in range(B)], axis=0)



# revision 16
# speedup vs baseline: 1.3028x; 1.0830x over previous
"""Transformer encoder layer (nn_Encoder) on 8 TRN2 NeuronCores.

Strategy: data-parallel over batch — B=8, one batch element per core, weights
replicated, no collectives. Per core a single Bass/Tile kernel computes the
whole layer; all large matmuls run with bf16 operands (full PE rate, less
power throttle than f32r) accumulating in fp32 PSUM.

Layout: attention runs in the "transposed domain" ([feature, tokens]) so every
weight matmul uses natural weight layouts; softmax over tokens-on-partitions is
handled by appending a ones-column to V (denominator lands in the ctx matmul's
extra output row, M=65). Per pair the 4 denominator rows are staged into one
[4, NS] tile, inverted with a single reciprocal, broadcast across partitions
with a tiny K=4 matmul against a selection matrix, and applied in one
full-width multiply per slice (deferred one pair to keep PE fed). Wo/FFN2
products land in the natural domain where both LayerNorms reduce along the
free dim; the residual add and the LN mean-sum share one DVE pass
(tensor_tensor_reduce) and the normalize runs on ACT via scale/bias. b2 is
folded into be1 on the host (with b1 -= W1^T b2 compensating FFN1). W1/W2/Wo
are prefetched into SBUF during the attention phase and stay resident, so the
FFN phase never waits on HBM. FFN2 runs si-major with per-si epilogues so the
final-tile epilogue is the only non-overlapped tail.

Self-contained: hardcodes B=8, S=1024, D=1024, H=16, FF=2048, 8 cores.
"""
import math
import numpy as np
import ml_dtypes
from contextlib import ExitStack

import concourse.bass as bass
import concourse.tile as tile
from concourse import bacc, mybir
from concourse import bass_utils
from concourse.masks import make_identity

B = 8
S = 1024
D = 1024
H = 16
FF = 2048
P = 128
HD = 64
EPS = 1e-5
f32 = mybir.dt.float32
f32r = mybir.dt.float32r
bf16 = mybir.dt.bfloat16
np_bf16 = ml_dtypes.bfloat16
AF = mybir.ActivationFunctionType
ALU = mybir.AluOpType
AX = mybir.AxisListType

USE_TTR = False        # fused residual-add + LN sum on DVE
USE_ACT_NORM = True   # (v-mu)*rstd on ACT via scale/bias APs

NP_ = H // 2          # head pairs
ST = S // P           # token tiles
DT = D // P
FT = FF // P
NS = 512              # token slice width (matmul free dim)
SL = S // NS
ND = 512              # feature slice width
DL = D // ND


def build_encoder(num_devices=8):
    scale = 1.0 / math.sqrt(HD)
    nc = bacc.Bacc("TRN2", target_bir_lowering=False, debug=False,
                   enable_asserts=True, num_devices=num_devices)

    dram = lambda n, sh, dt: nc.dram_tensor(n, sh, dt, kind="ExternalInput").ap()
    xT_d = dram("xT", [D, S], bf16)
    vones_d = dram("vones", [P, H], bf16)
    sel_d = dram("sel", [SL, 4, P], f32r)
    x_d = dram("x", [S, D], f32)
    wq_d = dram("Wq", [NP_, P, DT, P], bf16)
    wk_d = dram("Wk", [NP_, P, DT, P], bf16)
    wv_d = dram("Wv", [D, D], bf16)
    wo_d = dram("Wo", [D, D], bf16)
    w1_d = dram("W1", [FT, P, DT, P], bf16)
    w2_d = dram("W2", [FF, D], bf16)
    bqc_d = dram("bqc", [P, NP_], f32)
    bkc_d = dram("bkc", [P, NP_], f32)
    b1c_d = dram("b1c", [P, FT], f32)
    bv_d = dram("bv", [D], f32)
    bo_d = dram("bo", [D], f32)
    g1_d = dram("g1", [D], f32)
    be1_d = dram("be1", [D], f32)
    g2_d = dram("g2", [D], f32)
    be2_d = dram("be2", [D], f32)
    out_d = nc.dram_tensor("out", [S, D], f32, kind="ExternalOutput").ap()

    with tile.TileContext(nc) as tc, ExitStack() as octx:
        const = octx.enter_context(tc.tile_pool(name="const", bufs=1))
        identity = const.tile([P, P], bf16, name="identity")
        make_identity(nc, identity)

        # long-lived weight pool (prefetched during attention) + h tiles
        pW = octx.enter_context(tc.tile_pool(name="pW", bufs=1))
        pH = octx.enter_context(tc.tile_pool(name="pH", bufs=1))

        def bcast_row(pool, name, src_row, width):
            r = pool.tile([1, width], f32, name=f"{name}_r", tag="bcr", bufs=1)
            nc.sync.dma_start(r[:], src_row[None, :])
            b = pool.tile([P, width], f32, name=f"{name}_b", tag=f"{name}_b")
            nc.gpsimd.partition_broadcast(b[:], r[:])
            return b

        # ctxT pool (attention -> Wo)
        pCtx_cm = tc.tile_pool(name="pCtx", bufs=1)
        pCtx = pCtx_cm.__enter__()

        # ---------------- attention scope ----------------
        with tc.tile_pool(name="pA", bufs=1) as pA, \
             tc.tile_pool(name="psA", bufs=1, space="PSUM") as psA:

            # pair-0 Q/K weights first so QK(0) matmuls start ASAP
            wq0 = pA.tile([P, DT * P], bf16, name="wq0", tag="wq", bufs=2)
            nc.sync.dma_start(wq0[:].rearrange("p (dt q) -> p dt q", q=P),
                              wq_d[0])
            wk0 = pA.tile([P, DT * P], bf16, name="wk0", tag="wk", bufs=2)
            nc.sync.dma_start(wk0[:].rearrange("p (dt q) -> p dt q", q=P),
                              wk_d[0])

            xt = []
            for d in range(DT):
                t = pA.tile([P, S], bf16, name=f"xt{d}", tag="xt", bufs=DT)
                nc.sync.dma_start(t[:], xT_d[d * P:(d + 1) * P, :])
                xt.append(t)

            # ---- V projection weights ----
            pExp_cm = tc.tile_pool(name="pExp", bufs=1)
            pExp = pExp_cm.__enter__()
            pV_cm = tc.tile_pool(name="pV", bufs=1)
            pV = pV_cm.__enter__()
            wv = []
            for d in range(DT):
                t = pV.tile([P, D], bf16, name=f"wv{d}", tag="wv", bufs=DT)
                nc.sync.dma_start(t[:], wv_d[d * P:(d + 1) * P, :])
                wv.append(t)

            # V65 tiles: [128 t, H*65] with ones columns at 65h+64
            v65 = []
            for t in range(ST):
                v = pA.tile([P, H * 65], bf16, name=f"v65_{t}", tag="v65",
                            bufs=ST)
                nc.sync.dma_start(
                    v.rearrange("p (h c) -> p h c", c=65)[:, :, 64:65],
                    vones_d[:, :, None])
                v65.append(v)

            # small consts (needed only after the first QK chains land)
            bqc = const.tile([P, NP_], f32, name="bqc")
            nc.sync.dma_start(bqc[:], bqc_d)
            bkc = const.tile([P, NP_], f32, name="bkc")
            nc.sync.dma_start(bkc[:], bkc_d)
            b1c = const.tile([P, FT], f32, name="b1c")
            nc.sync.dma_start(b1c[:], b1c_d)
            selt = const.tile([4, SL * P], f32r, name="selt")
            for sl in range(SL):
                nc.sync.dma_start(selt[:, sl * P:(sl + 1) * P], sel_d[sl])
            bv_b = bcast_row(pA, "bv", bv_d, D)

            # Wo prefetch (needed right after attention)
            wo = []
            for p in range(NP_):
                t = pW.tile([P, D], bf16, name=f"wo{p}", tag="wo", bufs=NP_)
                nc.sync.dma_start(t[:], wo_d[p * P:(p + 1) * P, :])
                wo.append(t)

            # W1 resident tiles; DMAs emitted later in the pair loop
            w1 = [pW.tile([P, DT * P], bf16, name=f"w1_{f}", tag="w1", bufs=FT)
                  for f in range(FT)]

            def emit_w1_loads():
                for f in range(FT):
                    nc.sync.dma_start(
                        w1[f][:].rearrange("p (dt q) -> p dt q", q=P),
                        w1_d[f])

            hpn = ND // HD
            v_state = {}

            def emit_v_chunk(hc):
                """Half-chunk hc of the V projection (chain = hc//2)."""
                chain = hc // 2
                part = hc % 2
                t, n = chain // DL, chain % DL
                if part == 0:
                    v_state[chain] = psA.tile(
                        [P, ND], f32, name=f"vps{t}_{n}", tag="vqk", bufs=2)
                ps = v_state[chain]
                for d in range(4 * part, 4 * part + 4):
                    nc.tensor.matmul(
                        ps[:], xt[d][:, t * P:(t + 1) * P],
                        wv[d][:, n * ND:(n + 1) * ND],
                        start=(d == 0), stop=(d == DT - 1))
                if part == 1:
                    dst = v65[t].rearrange("p (h c) -> p h c", c=65)[
                        :, n * hpn:(n + 1) * hpn, 0:64]
                    srcv = ps[:].rearrange("p (h k) -> p h k", k=HD)
                    bvs = bv_b[:, n * ND:(n + 1) * ND].rearrange(
                        "p (h k) -> p h k", k=HD)
                    nc.vector.tensor_add(dst, srcv, bvs)

            # ---- attention per head pair ----
            ctxT = [pCtx.tile([P, S], bf16, name=f"ctxT{p}", tag="ctxT",
                              bufs=NP_) for p in range(NP_)]

            def emit_normalize(p, ctxU, den4):
                """Deferred softmax-normalize of pair p's ctx."""
                den4r = pA.tile([4, NS], f32r, name=f"den4r_{p}", tag="den4r",
                                bufs=2)
                with nc.allow_low_precision("softmax denom recip in f32r"):
                    nc.vector.reciprocal(den4r[:], den4[:])
                for sl in range(SL):
                    rcb = psA.tile([P, NS], f32, name=f"rcb{p}_{sl}",
                                   tag="vqk", bufs=2)
                    nc.tensor.matmul(rcb[:], selt[:, sl * P:(sl + 1) * P],
                                     den4r[:], start=True, stop=True)
                    nc.vector.tensor_mul(
                        ctxT[p][:, sl * NS:(sl + 1) * NS],
                        ctxU[:, sl * NS:(sl + 1) * NS], rcb[:])

            def emit_qk_chain_part(p, chain, part, state):
                """Emit 4 of the 8 accumulation matmuls of QK chain
                (chain: 0..3 = Q-sl0, Q-sl1, K-sl0, K-sl1) for pair p."""
                wt, bc, dst = state["ops"][chain // 2]
                sl = chain % 2
                if part == 0:
                    state[chain] = psA.tile(
                        [P, NS], f32, name=f"qk{p}_{chain}", tag="vqk", bufs=2)
                ps = state[chain]
                for d in range(4 * part, 4 * part + 4):
                    nc.tensor.matmul(
                        ps[:], wt[:, d * P:(d + 1) * P],
                        xt[d][:, sl * NS:(sl + 1) * NS],
                        start=(d == 0), stop=(d == DT - 1))
                if part == 1:
                    nc.vector.tensor_scalar(
                        out=dst[:, sl * NS:(sl + 1) * NS], in0=ps[:],
                        scalar1=bc[:, p:p + 1], scalar2=None, op0=ALU.add)

            def make_qk_state(p):
                if p == 0:
                    wqt, wkt = wq0, wk0
                else:
                    wqt = pA.tile([P, DT * P], bf16, name=f"wq{p}", tag="wq",
                                  bufs=2)
                    nc.sync.dma_start(
                        wqt[:].rearrange("p (dt q) -> p dt q", q=P),
                        wq_d[p])
                    wkt = pA.tile([P, DT * P], bf16, name=f"wk{p}", tag="wk",
                                  bufs=2)
                    nc.sync.dma_start(
                        wkt[:].rearrange("p (dt q) -> p dt q", q=P),
                        wk_d[p])
                qt = pA.tile([P, S], bf16, name=f"qt{p}", tag="qt", bufs=2)
                kt = pA.tile([P, S], bf16, name=f"kt{p}", tag="kt", bufs=2)
                return {"ops": ((wqt, bqc, qt), (wkt, bkc, kt)),
                        "qt": qt, "kt": kt}

            LAG = 2
            qk_state = make_qk_state(0)
            for chain in range(4):
                for part in range(2):
                    emit_qk_chain_part(0, chain, part, qk_state)

            pending = None
            for p in range(NP_):
                if p == 2:
                    emit_w1_loads()
                qt, kt = qk_state["qt"], qk_state["kt"]
                next_state = make_qk_state(p + 1) if p + 1 < NP_ else None

                ctxU = pA.tile([P, S], f32, name=f"ctxU{p}", tag="ctxU",
                               bufs=2)
                den4 = pA.tile([4, NS], f32, name=f"den4_{p}", tag="den4",
                               bufs=2)

                def emit_scores(sl, t, expt):
                    ps = psA.tile([P, 2 * NS], f32, name=f"sc{t}_{sl}",
                                  tag="sc", bufs=2)
                    for h in range(2):
                        nc.tensor.matmul(
                            ps[:, h * NS:(h + 1) * NS],
                            kt[h * HD:(h + 1) * HD, t * P:(t + 1) * P],
                            qt[h * HD:(h + 1) * HD, sl * NS:(sl + 1) * NS],
                            start=True, stop=True,
                            tile_position=(h * HD, 0))
                    e = pExp.tile([P, 2 * NS], bf16, name=f"e{t}_{sl}",
                                  tag="exp", bufs=3)
                    nc.scalar.activation(e[:], ps[:], AF.Exp, scale=scale)
                    expt[t] = e

                def emit_ctx(sl, tt, cps, expt):
                    for h in range(2):
                        lhs = v65[tt][:, (2 * p + h) * 65:(2 * p + h) * 65 + 65]
                        nc.tensor.matmul(
                            cps[h][0:65, :], lhs,
                            expt[tt][:, h * NS:(h + 1) * NS],
                            start=(tt == 0), stop=(tt == ST - 1))

                def emit_evict(sl, cps):
                    for h in range(2):
                        ps = cps[h]
                        stage = pA.tile([65, NS], f32, name=f"stg{h}{sl}",
                                        tag="rc", bufs=2)
                        nc.vector.tensor_copy(stage[64:65, :], ps[64:65, :])
                        nc.sync.dma_start(
                            den4[h * SL + sl:h * SL + sl + 1, :],
                            stage[64:65, :])
                        if h == 0:
                            nc.vector.tensor_copy(
                                ctxU[0:HD, sl * NS:(sl + 1) * NS],
                                ps[0:HD, :])
                        else:
                            tmp = pA.tile([HD, NS], f32, name=f"ctmp{sl}",
                                          tag="ctmp", bufs=2)
                            nc.vector.tensor_copy(tmp[:], ps[0:HD, :])
                            nc.sync.dma_start(
                                ctxU[HD:P, sl * NS:(sl + 1) * NS], tmp[:])

                expt0 = {}
                cps0 = [psA.tile([P, NS], f32, name=f"cps{h}_0", tag="ctx",
                                 bufs=2) for h in range(2)]
                expt1 = {}
                cps1 = [psA.tile([P, NS], f32, name=f"cps{h}_1", tag="ctx",
                                 bufs=2) for h in range(2)]
                if p == 0:
                    # A: scores(sl0) + the whole V projection interleaved
                    for t in range(ST):
                        emit_scores(0, t, expt0)
                        for hc in range(4 * t, 4 * t + 4):
                            emit_v_chunk(hc)
                    # B: scores(sl1) + lagged ctx(sl0)
                    for t in range(ST + LAG):
                        if t < ST:
                            emit_scores(1, t, expt1)
                        if t >= LAG:
                            emit_ctx(0, t - LAG, cps0, expt0)
                    emit_evict(0, cps0)
                    # C: ctx(sl1) + QK(1) chunks
                    for t in range(ST):
                        emit_ctx(1, t, cps1, expt1)
                        if next_state is not None:
                            emit_qk_chain_part(p + 1, t // 2, t % 2,
                                               next_state)
                    emit_evict(1, cps1)
                    pV_cm.__exit__(None, None, None)
                else:
                    # A: scores(sl0) + QK(p+1) chunks 0-3 + lagged ctx(sl0)
                    for t in range(ST + LAG):
                        if t < ST:
                            emit_scores(0, t, expt0)
                            if next_state is not None and t < 4:
                                emit_qk_chain_part(p + 1, t // 2, t % 2,
                                                   next_state)
                        if t >= LAG:
                            emit_ctx(0, t - LAG, cps0, expt0)
                    emit_evict(0, cps0)
                    if pending is not None:
                        emit_normalize(*pending)
                    # B: scores(sl1) + QK(p+1) chunks 4-7 + lagged ctx(sl1)
                    for t in range(ST + LAG):
                        if t < ST:
                            emit_scores(1, t, expt1)
                            if next_state is not None and t < 4:
                                emit_qk_chain_part(p + 1, (t + 4) // 2,
                                                   t % 2, next_state)
                        if t >= LAG:
                            emit_ctx(1, t - LAG, cps1, expt1)
                    emit_evict(1, cps1)
                pending = (p, ctxU, den4)
                qk_state = next_state
            emit_normalize(*pending)
            pExp_cm.__exit__(None, None, None)

        # ---------------- Wo + LN1 scope ----------------
        # hn = LN1(x + ctx@Wo + bo) * g1 + (be1 + b2)   [b2 folded on host;
        # b1 of FFN1 compensated with -W1^T b2]
        h_nat = []
        ht = [pH.tile([P, S], bf16, name=f"ht{d}", tag="ht", bufs=DT)
              for d in range(DT)]
        with tc.tile_pool(name="pWo", bufs=1) as pWo, \
             tc.tile_pool(name="psW", bufs=1, space="PSUM") as psW:
            bo_b = bcast_row(pWo, "bo", bo_d, D)
            g1_b = bcast_row(pWo, "g1", g1_d, D)
            be1_b = bcast_row(pWo, "be1", be1_d, D)

            # residual tiles: x + bo, prefetched for all si
            xns = []
            for si in range(ST):
                xn = pWo.tile([P, D], f32, name=f"xn{si}", tag="xn", bufs=ST)
                nc.sync.dma_start(xn[:], x_d[si * P:(si + 1) * P, :])
                nc.vector.tensor_add(xn[:], xn[:], bo_b[:])
                xns.append(xn)

            for si in range(ST):
                pss = [psW.tile([P, ND], f32, name=f"c{si}_{n}", tag="c",
                                bufs=4) for n in range(DL)]
                for p in range(NP_):
                    for n in range(DL):
                        nc.tensor.matmul(
                            pss[n][:], ctxT[p][:, si * P:(si + 1) * P],
                            wo[p][:, n * ND:(n + 1) * ND],
                            start=(p == 0), stop=(p == NP_ - 1))
                # fused residual add + LN1 sum
                st = pWo.tile([P, 8], f32, name=f"st1_{si}", tag="st1", bufs=4)
                s1a = st[:, 0:1]; s1b = st[:, 1:2]; s1 = st[:, 2:3]
                s2 = st[:, 3:4]; mu = st[:, 4:5]; var = st[:, 5:6]
                rstd = st[:, 6:7]; nm = st[:, 7:8]
                v = pWo.tile([P, D], f32, name=f"v{si}", tag="v", bufs=4)
                scr = pWo.tile([P, D], f32, name=f"scr{si}", tag="scr", bufs=3)
                if USE_TTR:
                    for n in range(DL):
                        nc.vector.tensor_tensor_reduce(
                            out=v[:, n * ND:(n + 1) * ND], in0=pss[n][:],
                            in1=xns[si][:, n * ND:(n + 1) * ND], scale=1.0,
                            scalar=0.0, op0=ALU.add, op1=ALU.add,
                            accum_out=(s1a if n == 0 else s1b))
                    nc.vector.tensor_add(s1, s1a, s1b)
                else:
                    for n in range(DL):
                        nc.vector.tensor_add(
                            v[:, n * ND:(n + 1) * ND], pss[n][:],
                            xns[si][:, n * ND:(n + 1) * ND])
                    nc.scalar.activation(scr[:], v[:], AF.Copy, accum_out=s1)
                nc.scalar.activation(scr[:], v[:], AF.Square, accum_out=s2)
                nc.vector.tensor_scalar_mul(mu, s1, 1.0 / D)
                nc.vector.tensor_scalar_mul(var, s2, 1.0 / D)
                nc.vector.tensor_mul(nm, mu, mu)
                nc.vector.tensor_sub(var, var, nm)
                nc.vector.tensor_scalar_add(var, var, EPS)
                nc.scalar.sqrt(var, var)
                nc.vector.reciprocal(rstd, var)
                nc.vector.tensor_mul(nm, mu, rstd)
                nc.vector.tensor_scalar_mul(nm, nm, -1.0)
                # vh = (v - mu) * rstd on ACT; then affine on DVE -> hn bf16
                hn = pH.tile([P, D], bf16, name=f"hn{si}", tag="hn", bufs=ST)
                if USE_ACT_NORM:
                    nc.scalar.activation(scr[:], v[:], AF.Identity,
                                         bias=nm, scale=rstd)
                else:
                    nc.vector.tensor_scalar(out=scr[:], in0=v[:], scalar1=mu,
                                            scalar2=rstd, op0=ALU.subtract,
                                            op1=ALU.mult)
                nc.vector.tensor_mul(scr[:], scr[:], g1_b[:])
                nc.vector.tensor_add(hn[:], scr[:], be1_b[:])
                h_nat.append(hn)
                # h^T transposes for this si; copybacks alternate DVE/ACT
                for dd in range(DT):
                    ps = psW.tile([P, P], bf16, name=f"tp{si}_{dd}", tag="tp",
                                  bufs=4)
                    nc.tensor.transpose(
                        ps[:], hn[:, dd * P:(dd + 1) * P], identity[:])
                    dst = ht[dd][:, si * P:(si + 1) * P]
                    if dd % 2 == 0:
                        nc.vector.tensor_copy(dst, ps[:])
                    else:
                        nc.scalar.copy(dst, ps[:])
        pCtx_cm.__exit__(None, None, None)

        # ---------------- FFN + LN2 scope ----------------
        # FFN1 over the whole S (W1 resident), then si-major FFN2 with per-si
        # epilogue (fused residual+sum, ACT normalize) so only the last si's
        # epilogue is exposed at the end.
        with tc.tile_pool(name="pF", bufs=1) as pF:
            g2_b = bcast_row(pF, "g2", g2_d, D)
            be2_b = bcast_row(pF, "be2", be2_d, D)

            # W2 streamed (once) during the FFN1 window
            w2 = []
            for f in range(FT):
                t = pF.tile([P, D], bf16, name=f"w2_{f}", tag="w2", bufs=FT)
                nc.sync.dma_start(t[:], w2_d[f * P:(f + 1) * P, :])
                w2.append(t)

            ut = []
            with tc.tile_pool(name="psU", bufs=1, space="PSUM") as psU:
                for f in range(FT):
                    u = pF.tile([P, S], bf16, name=f"ut{f}", tag="ut",
                                bufs=FT)
                    for hf in range(SL):
                        ps = psU.tile([P, NS], f32, name=f"u{f}_{hf}",
                                      tag="u", bufs=4)
                        for d in range(DT):
                            nc.tensor.matmul(
                                ps[:], w1[f][:, d * P:(d + 1) * P],
                                ht[d][:, hf * NS:(hf + 1) * NS],
                                start=(d == 0), stop=(d == DT - 1))
                        nc.scalar.activation(u[:, hf * NS:(hf + 1) * NS],
                                             ps[:], AF.Relu,
                                             bias=b1c[:, f:f + 1])
                    ut.append(u)

            with tc.tile_pool(name="psY", bufs=1, space="PSUM") as psY:
                for si in range(ST):
                    pss = [psY.tile([P, ND], f32, name=f"y{si}_{n}", tag="y",
                                    bufs=4) for n in range(DL)]
                    for f in range(FT):
                        for n in range(DL):
                            nc.tensor.matmul(
                                pss[n][:],
                                ut[f][:, si * P:(si + 1) * P],
                                w2[f][:, n * ND:(n + 1) * ND],
                                start=(f == 0), stop=(f == FT - 1))
                    # fused epilogue: v = y + hn; s1 alongside
                    st = pF.tile([P, 8], f32, name=f"st2_{si}", tag="st2",
                                 bufs=4)
                    s1a = st[:, 0:1]; s1b = st[:, 1:2]; s1 = st[:, 2:3]
                    s2 = st[:, 3:4]; mu = st[:, 4:5]; var = st[:, 5:6]
                    rstd = st[:, 6:7]; nm = st[:, 7:8]
                    v = pF.tile([P, D], f32, name=f"v2_{si}", tag="v2",
                                bufs=4)
                    scr = pF.tile([P, D], f32, name=f"scr2_{si}", tag="scr2",
                                  bufs=3)
                    if USE_TTR:
                        for n in range(DL):
                            nc.vector.tensor_tensor_reduce(
                                out=v[:, n * ND:(n + 1) * ND], in0=pss[n][:],
                                in1=h_nat[si][:, n * ND:(n + 1) * ND],
                                scale=1.0, scalar=0.0, op0=ALU.add,
                                op1=ALU.add,
                                accum_out=(s1a if n == 0 else s1b))
                        nc.vector.tensor_add(s1, s1a, s1b)
                    else:
                        for n in range(DL):
                            nc.vector.tensor_add(
                                v[:, n * ND:(n + 1) * ND], pss[n][:],
                                h_nat[si][:, n * ND:(n + 1) * ND])
                        nc.scalar.activation(scr[:], v[:], AF.Copy,
                                             accum_out=s1)
                    nc.scalar.activation(scr[:], v[:], AF.Square,
                                         accum_out=s2)
                    nc.vector.tensor_scalar_mul(mu, s1, 1.0 / D)
                    nc.vector.tensor_scalar_mul(var, s2, 1.0 / D)
                    nc.vector.tensor_mul(nm, mu, mu)
                    nc.vector.tensor_sub(var, var, nm)
                    nc.vector.tensor_scalar_add(var, var, EPS)
                    nc.scalar.sqrt(var, var)
                    nc.vector.reciprocal(rstd, var)
                    nc.vector.tensor_mul(nm, mu, rstd)
                    nc.vector.tensor_scalar_mul(nm, nm, -1.0)
                    o = pF.tile([P, D], f32, name=f"o{si}", tag="o", bufs=3)
                    if USE_ACT_NORM:
                        nc.scalar.activation(scr[:], v[:], AF.Identity,
                                             bias=nm, scale=rstd)
                    else:
                        nc.vector.tensor_scalar(out=scr[:], in0=v[:],
                                                scalar1=mu, scalar2=rstd,
                                                op0=ALU.subtract,
                                                op1=ALU.mult)
                    nc.vector.tensor_mul(scr[:], scr[:], g2_b[:])
                    nc.vector.tensor_add(o[:], scr[:], be2_b[:])
                    nc.sync.dma_start(out_d[si * P:(si + 1) * P, :], o[:])

    nc.compile()
    return nc


def pack_core_inputs(x_b, shared):
    """Per-core input map: batch element x_b + shared (prepacked) weights."""
    m = dict(shared)
    m["x"] = np.ascontiguousarray(x_b, dtype=np.float32)
    m["xT"] = np.ascontiguousarray(x_b.T.astype(np_bf16))
    return m


def pack_shared(Wq, bq, Wk, bk, Wv, bv, Wo, bo, ln1_g, ln1_b, W1, b1, W2, b2,
                ln2_g, ln2_b):
    """Host-side layout packing of the replicated weights (pure layout +
    exact algebraic folds: be1 += b2 with b1 -= W1^T b2)."""
    f = np.float32
    Wq = np.asarray(Wq, dtype=f); Wk = np.asarray(Wk, dtype=f)
    Wv = np.asarray(Wv, dtype=f)
    W1 = np.asarray(W1, dtype=f); W2 = np.asarray(W2, dtype=f)
    b1 = np.asarray(b1, dtype=f); b2 = np.asarray(b2, dtype=f)
    ln1_b = np.asarray(ln1_b, dtype=f)
    # [pair, dp, dt, q] so per-pair weight DMA is contiguous
    pack_qk = lambda W: np.ascontiguousarray(
        W.reshape(D, H * HD).reshape(DT, P, NP_, P).transpose(
            2, 1, 0, 3)).astype(np_bf16)
    sel = np.zeros((SL, 4, P), dtype=f)
    for sl in range(SL):
        for m in range(P):
            sel[sl, (m // HD) * SL + sl, m] = 1.0
    # fold b2 into be1; compensate FFN1 bias (exact, pre-ReLU linearity)
    be1_f = ln1_b + b2
    b1_f = (b1.astype(np.float64) -
            W1.astype(np.float64).T @ b2.astype(np.float64)).astype(f)
    return {
        "vones": np.ones((P, H), dtype=np_bf16),
        "sel": sel,
        "Wq": pack_qk(Wq), "Wk": pack_qk(Wk),
        "Wv": np.ascontiguousarray(Wv.reshape(D, D)).astype(np_bf16),
        "Wo": np.ascontiguousarray(Wo, dtype=f).astype(np_bf16),
        "W1": np.ascontiguousarray(
            W1.reshape(DT, P, FT, P).transpose(2, 1, 0, 3)).astype(np_bf16),
        "W2": np.ascontiguousarray(W2).astype(np_bf16),
        "bqc": np.ascontiguousarray(np.asarray(bq, f).reshape(NP_, P).T),
        "bkc": np.ascontiguousarray(np.asarray(bk, f).reshape(NP_, P).T),
        "b1c": np.ascontiguousarray(b1_f.reshape(FT, P).T),
        "bv": np.ascontiguousarray(np.asarray(bv, f).reshape(D)),
        "bo": np.ascontiguousarray(bo, dtype=f),
        "g1": np.ascontiguousarray(ln1_g, dtype=f),
        "be1": np.ascontiguousarray(be1_f),
        "g2": np.ascontiguousarray(ln2_g, dtype=f),
        "be2": np.ascontiguousarray(ln2_b, dtype=f),
    }


_NC_CACHE = {}


def get_nc():
    if "nc" not in _NC_CACHE:
        _NC_CACHE["nc"] = build_encoder(num_devices=8)
    return _NC_CACHE["nc"]


def kernel(x, Wq, bq, Wk, bk, Wv, bv, Wo, bo, ln1_g, ln1_b, W1, b1, W2, b2,
           ln2_g, ln2_b):
    x = np.asarray(x)
    assert x.shape == (B, S, D)
    shared = pack_shared(Wq, bq, Wk, bk, Wv, bv, Wo, bo, ln1_g, ln1_b,
                         W1, b1, W2, b2, ln2_g, ln2_b)
    in_maps = [pack_core_inputs(x[b], shared) for b in range(B)]
    nc = get_nc()
    res = bass_utils.run_bass_kernel_spmd(
        nc, in_maps, core_ids=list(range(B)), trace=False)
    return np.stack([res.results[b]["out"] for b in range(B)], axis=0)
